# revision 18
# baseline (speedup 1.0000x reference)
"""Trainium2 Bass kernel for nn_Encoder_78649441124984.

Encoder: pos_emb + 4x(sepconv+res) + MHA(+res) + ffc(+res).
Sharding: data-parallel over batch, 8 cores x 4 batch elements, all
parameters replicated; no collectives.

On-device layout: activations kept transposed as [feature, time] tiles
([128, 512] SBUF tiles, feature on partitions).

Speed structure (v1):
 - sepconv (depthwise + pointwise) runs in fp8e4m3 with DoubleRow perf
   mode (2 contraction groups per pass at 0.5 cycles/row).  Weight
   quantization error is cancelled by a hi/lo split: W ~= fp8(W) +
   fp8(W - fp8(W)), two DoubleRow passes.  Depthwise pairs taps
   (s, s+4) as the two groups of one DoubleRow matmul over a
   zero-padded fp8 input tile.
 - everything else (qkv, scores, av, out/ffc proj) runs in
   float32r: at moving size >= 256 the PE runs f32r at bf16 speed,
   so this is free accuracy.  vaug/expt/anorm/owT run bf16 (their
   quantization is harmless); the residual stream stays f32r.
"""
import sys

sys.path.insert(0, "/opt/trn_rl_repo")

import numpy as np
import ml_dtypes

import concourse.bass as bass
import concourse.mybir as mybir
import concourse.tile as tile
from concourse import bacc
from concourse.ap import AP as APc
from concourse.bass_utils import run_bass_kernel_spmd

F32 = mybir.dt.float32
F32R = mybir.dt.float32r
BF16 = mybir.dt.bfloat16
FP8 = mybir.dt.float8e4
I32 = mybir.dt.int32
U8 = mybir.dt.uint8
AF = mybir.ActivationFunctionType
ALU = mybir.AluOpType
DR = mybir.MatmulPerfMode.DoubleRow
FP8NP = ml_dtypes.float8_e4m3
BF16NP = ml_dtypes.bfloat16

D = 500
H = 10
HD = 50
B, T = 32, 512
K = 7
NC_ = 8
BS = B // NC_          # batch shard per core
DP = 512               # padded feature dim
CT = 4                 # feature tiles (4 x 128 = 512 >= 500)
HP = 640               # padded head dim total (10 heads x 64 slots)
VW = 768               # v-proj rhs width (640 padded to 768 so the
                       # second psum piece has moving size 256)
XW = 520               # padded conv input tile width (4 + 512 + 4)


def _rows(ct):
    return min(128, D - 128 * ct)


def _head_col(h):
    return 128 * (h // 2) + 64 * (h % 2)


def _q8(a):
    return a.astype(FP8NP).astype(np.float32)


def build_host_consts(dw, pw, db, pb, in_w, in_b, out_w, out_b, ffc_w, ffc_b):
    """Pack all weights into device layouts. dw: [4][D,1,K], pw: [4][D,D]."""
    c = {}
    # ---------------- fp8 wall: depthwise diag pairs + pointwise ----------
    # depthwise: per (layer, block): 8 DoubleRow lhsT of [128, 2, 128]:
    # halves (hi, lo) x tap-pairs j=0..3 with taps (j-3, j+1); tap 4 = 0.
    # pointwise: per layer: halves (hi, lo) x ct-pairs c=0,1 of
    # [128, 2, 512]: group g holds pwT rows of ct=2c+g.
    w8_secs = []
    for l in range(4):
        dwf = dw[l][:, 0, :]                      # [D, K]
        dwp = np.zeros((DP, K + 2), np.float32)   # taps -3..3 plus zero tap 4
        dwp[:D, :K] = dwf
        hi = _q8(dwp)
        lo = dwp - hi                             # fp8 cast below
        diag = np.zeros((2, CT, 4, 128, 2, 128), np.float32)
        for half, w in ((0, hi), (1, lo)):
            for blk in range(CT):
                for j in range(4):
                    for g, tap in ((0, j), (1, j + 4)):
                        np.fill_diagonal(diag[half, blk, j, :, g, :],
                                         w[128 * blk:128 * blk + 128, tap])
        w8_secs.append((f"diag{l}", diag.reshape(2, CT, 4, 128, 256)
                        .transpose(3, 1, 0, 2, 4).reshape(128, -1)))
        # cols layout: blk-major, then half, then j, then [2x128]
        pwT = np.zeros((DP, DP), np.float32)
        pwT[:D, :D] = pw[l].T
        pwh = _q8(pwT)
        pwl = pwT - pwh
        pk = np.zeros((2, 2, 128, 2, DP), np.float32)
        for half, w in ((0, pwh), (1, pwl)):
            for cpair in range(2):
                for g in range(2):
                    ct = 2 * cpair + g
                    pk[half, cpair, :, g, :] = w[128 * ct:128 * ct + 128, :]
        # layout: half-major, then cpair, then [128, 2, 512] -> [128, 2048]
        w8_secs.append((f"pw{l}", pk.transpose(2, 0, 1, 3, 4).reshape(128, -1)))
    offs8 = {}
    w = 0
    parts = []
    for name, arr in w8_secs:
        offs8[name] = w
        w += arr.shape[1]
        parts.append(arr)
    c["wall8"] = np.concatenate(parts, 1).astype(FP8NP)
    c["_offs8"] = offs8

    # ---------------- f32r wall: qkv in-proj, v-proj, G ------------------
    scale = HD ** -0.5
    inwT = np.zeros((DP, 2 * HP), np.float32)
    inb_cols = np.zeros((128, 10), np.float32)
    for h in range(H):
        base = _head_col(h)
        qrows = slice(100 * (h // 2) + 50 * (h % 2),
                      100 * (h // 2) + 50 * (h % 2) + 50)
        inwT[:D, base:base + 50] = in_w[qrows, :].T * scale
        inb_cols[base % 128:base % 128 + 50, h // 2] = in_b[qrows] * scale
        krows = slice(500 + qrows.start, 500 + qrows.stop)
        inwT[:D, HP + base:HP + base + 50] = in_w[krows, :].T
        inb_cols[base % 128:base % 128 + 50, 5 + h // 2] = in_b[krows]
    wv = np.zeros((DP, VW), np.float32)
    crow = np.zeros((1, HP), np.float32)
    for h in range(H):
        base = _head_col(h)
        vrows = slice(1000 + 50 * h, 1000 + 50 * h + 50)
        wv[:D, base:base + 50] = in_w[vrows, :].T
        wv[D, base + 50] = 1.0    # ones column via the stream's 1.0 row
        crow[0, base:base + 50] = in_b[vrows]
        crow[0, base + 50] = 1.0
    G = np.zeros((5 * 128, H), np.float32)
    E = np.zeros((5 * H, 128), np.float32)
    for p in range(5):
        G[128 * p + 50, 2 * p] = 1.0
        G[128 * p + 114, 2 * p + 1] = 1.0
        E[H * p + 2 * p, 0:50] = 1.0
        E[H * p + 2 * p + 1, 64:114] = 1.0
    f32_secs = [("inwT", _rt(inwT)), ("wv", _rt(wv)),
                ("ffcT", _rt(np.pad(ffc_w.T, ((0, DP - D), (0, DP - D)))))]
    offsf = {}
    w = 0
    parts = []
    for name, arr in f32_secs:
        offsf[name] = w
        w += arr.shape[1]
        parts.append(arr)
    c["wallf"] = np.concatenate(parts, 1).astype(np.float32)
    c["_offsf"] = offsf

    # ---------------- bf16 wall: out-proj + ffc --------------------------
    owT = np.zeros((HP, DP), np.float32)
    for h in range(H):
        base = _head_col(h)
        owT[base:base + 50, :D] = out_w[:, 50 * h:50 * h + 50].T
    b16_secs = [("owT", _rt(owT)), ("G", _rt(G))]
    offsb = {}
    w = 0
    parts = []
    for name, arr in b16_secs:
        offsb[name] = w
        w += arr.shape[1]
        parts.append(arr)
    c["wallb"] = np.concatenate(parts, 1).astype(BF16NP)
    c["_offsb"] = offsb

    # ---------------- small f32 tensors ----------------------------------
    sm = np.concatenate(
        [inb_cols,
         np.pad(out_b, (0, DP - D)).reshape(CT, 128).T,
         np.pad(ffc_b, (0, DP - D)).reshape(CT, 128).T,
         np.concatenate([np.pad(db[l], (0, DP - D)).reshape(CT, 128).T
                         for l in range(4)], 1),
         np.concatenate([np.pad(pb[l], (0, DP - D)).reshape(CT, 128).T
                         for l in range(4)], 1)], 1).astype(np.float32)
    c["smallf"] = sm
    c["E_all"] = np.ascontiguousarray(
        np.concatenate([E[10 * p:10 * (p + 1), :] for p in range(5)], 1)
    ).astype(BF16NP)
    c["crow"] = crow.astype(np.float32)
    c["onesrow"] = np.ones((1, T), np.float32)
    half = D // 2
    inv = np.exp(np.arange(half, dtype=np.float64) * (-np.log(10000.0) / (half - 1)))
    pos = np.arange(1, T + 1, dtype=np.float64)
    ang = pos[None, :] * inv[:, None]
    peT = np.zeros((DP, T), np.float32)
    peT[:half, :] = np.sin(ang)
    peT[half:D, :] = np.cos(ang)
    c["peTp"] = _rt(peT).astype(BF16NP)
    return c


def _rt(a):
    """Repack row-tiled [n*128, C] -> [128, n*C] (tile ct at cols ct*C)."""
    n = a.shape[0] // 128
    return a.reshape(n, 128, a.shape[1]).transpose(1, 0, 2).reshape(128, -1)


def trace_program(consts, mask_any, bias_any, pad_any):
    """Build the SPMD Bass program (same for every core)."""
    nc = bacc.Bacc("TRN2", target_bir_lowering=False, debug=False,
                   num_devices=NC_)

    import os
    dbg = os.environ.get("BASSDBG") == "1"
    xT_d = nc.dram_tensor("xT", [BS, D, T], F32, kind="ExternalInput")
    orix_d = nc.dram_tensor("orix", [BS, T], I32, kind="ExternalInput")
    xmask_d = nc.dram_tensor("xmask", [BS, T], U8, kind="ExternalInput")
    out_d = nc.dram_tensor("out", [BS, D, T], F32, kind="ExternalOutput")
    dbg_d = None
    if dbg:
        dbg_d = {}
        for n in ("pos", "l1", "l4", "qk0", "qk5", "x2"):
            dbg_d[n] = nc.dram_tensor(f"dbg_{n}", [512, T], F32R, kind="ExternalOutput")
        for n in ("vaug", "abuf", "anorm", "mbc"):
            dbg_d[n] = nc.dram_tensor(f"dbg_{n}", [512, T], BF16, kind="ExternalOutput")

    wd = {"_offs8": consts["_offs8"], "_offsf": consts["_offsf"],
          "_offsb": consts["_offsb"]}
    dts = {"wall8": FP8, "wallf": F32R, "wallb": BF16, "smallf": F32,
           "E_all": BF16, "crow": F32, "peTp": BF16, "onesrow": F32R}
    for name, arr in consts.items():
        if name.startswith("_"):
            continue
        wd[name] = nc.dram_tensor(name, list(arr.shape), dts[name],
                                  kind="ExternalInput")

    with tile.TileContext(nc, num_cores=NC_) as tc:
        wd["_dbg"] = dbg_d
        _trace_body(nc, tc, wd, xT_d, orix_d, xmask_d, out_d,
                    mask_any, bias_any, pad_any)
    nc.finalize()
    return nc


def _trace_body(nc, tc, wd, xT_d, orix_d, xmask_d, out_d,
                mask_any, bias_any, pad_any):
    from contextlib import ExitStack
    ctx = ExitStack()
    with ctx:
        wpool = ctx.enter_context(tc.tile_pool(name="w", bufs=1))
        offs8 = wd["_offs8"]
        offsf = wd["_offsf"]
        offsb = wd["_offsb"]
        W8 = wd["wall8"].shape[1]
        WF = wd["wallf"].shape[1]
        WB = wd["wallb"].shape[1]
        wall8 = wpool.tile([128, W8], FP8, tag="wall8", name="wall8")
        wallf = wpool.tile([128, WF], F32R, tag="wallf", name="wallf")
        wallb = wpool.tile([128, WB], BF16, tag="wallb", name="wallb")
        # urgent small constants on the ACT ring
        peTp = wpool.tile([128, CT * T], BF16, tag="peTp", name="peTp")
        nc.scalar.dma_start(peTp[:], wd["peTp"][:])
        smallf = wpool.tile([128, 50], F32, tag="smallf", name="smallf")
        nc.scalar.dma_start(smallf[:], wd["smallf"][:])
        E_t = wpool.tile([H, 5 * 128], BF16, tag="E_t", name="E_t")
        nc.scalar.dma_start(E_t[:], wd["E_all"][:])
        crow_t = None
        if bias_any:
            crow_t = wpool.tile([1, HP], F32, tag="crow", name="crow")
            nc.scalar.dma_start(crow_t[:], wd["crow"][:])
        # walls on the SP ring, section-by-section in first-use order
        def sec_dmas(tile_t, dram, offd, order, width):
            sw = {}
            for s in offd:
                nxt = [offd[t] for t in offd if offd[t] > offd[s]]
                sw[s] = (min(nxt) if nxt else width) - offd[s]
            for s in order:
                nc.sync.dma_start(tile_t[:, offd[s]:offd[s] + sw[s]],
                                  dram[:, offd[s]:offd[s] + sw[s]])
        o8 = []
        for l in range(4):
            o8 += [f"diag{l}", f"pw{l}"]
        sec_dmas(wall8, wd["wall8"], offs8, o8, W8)
        sec_dmas(wallf, wd["wallf"], offsf, ["inwT", "wv", "ffcT"], WF)
        sec_dmas(wallb, wd["wallb"], offsb, ["owT", "G"], WB)
        C_t = None
        if bias_any:
            C_t = wpool.tile([128, HP], F32, tag="C", name="C")
            nc.gpsimd.partition_broadcast(C_t[:], crow_t[:])

        # weight-slice helpers -------------------------------------------
        def dw_lhsT(l, blk, half, j):
            off = offs8[f"diag{l}"] + blk * 2048 + half * 1024 + j * 256
            base = wall8[:, off:off + 256]
            return APc(base.tensor, base.offset,
                       [list(base.ap[0]), [128, 2], [1, 128]])

        def pw_lhsT(l, half, cpair, ot):
            off = offs8[f"pw{l}"] + half * 2048 + cpair * 1024 + 128 * ot
            base = wall8[:, off:off + 1]
            return APc(base.tensor, base.offset,
                       [list(base.ap[0]), [512, 2], [1, 128]])

        inwT = [wallf[:, offsf["inwT"] + 2 * HP * ct:
                       offsf["inwT"] + 2 * HP * (ct + 1)] for ct in range(CT)]
        wv = [wallf[:, offsf["wv"] + VW * ct:offsf["wv"] + VW * (ct + 1)]
              for ct in range(CT)]
        G = [wallb[:, offsb["G"] + H * p:offsb["G"] + H * (p + 1)]
             for p in range(5)]
        owT = [wallb[:, offsb["owT"] + DP * p:offsb["owT"] + DP * (p + 1)]
               for p in range(5)]
        ffcT = [wallf[:, offsf["ffcT"] + DP * ct:offsf["ffcT"] + DP * (ct + 1)]
                for ct in range(CT)]
        E = [E_t[:, 128 * p:128 * (p + 1)] for p in range(5)]
        peT = [peTp[:, T * ct:T * (ct + 1)] for ct in range(CT)]
        inb_cols = smallf[:, 0:10]
        outb_col = smallf[:, 10:14]
        ffcb_col = smallf[:, 14:18]
        db_cols = [smallf[:, 18 + CT * l:18 + CT * (l + 1)] for l in range(4)]
        pb_cols = [smallf[:, 34 + CT * l:34 + CT * (l + 1)] for l in range(4)]

        # ---- per-batch-element pools ----
        xpool = ctx.enter_context(tc.tile_pool(name="x", bufs=3))
        f8pool = ctx.enter_context(tc.tile_pool(name="f8", bufs=2))
        mpool = ctx.enter_context(tc.tile_pool(name="m", bufs=2))
        qkpool = ctx.enter_context(tc.tile_pool(name="qk", bufs=1))
        epool = ctx.enter_context(tc.tile_pool(name="e", bufs=2))
        apool = ctx.enter_context(tc.tile_pool(name="a", bufs=2))
        opool = ctx.enter_context(tc.tile_pool(name="o", bufs=2))
        pp = ctx.enter_context(tc.tile_pool(name="pp", bufs=6, space="PSUM"))
        pa = ctx.enter_context(tc.tile_pool(name="pa", bufs=1, space="PSUM"))

        gens = [
            _trace_batch(nc, tc, b, wd, xT_d, orix_d, xmask_d, out_d,
                         dw_lhsT, pw_lhsT, inwT, wv, owT, ffcT, peT, G, E, C_t,
                         inb_cols, outb_col, ffcb_col, db_cols, pb_cols,
                         xpool, f8pool, mpool, qkpool, epool, apool, opool,
                         pp, pa, mask_any, bias_any, pad_any)
            for b in range(BS)
        ]
        done = [False] * BS
        last = ["f"] * BS

        def step(i):
            try:
                last[i] = next(gens[i])
            except StopIteration:
                done[i] = True

        import os
        if os.environ.get("NOILV") == "1":
            for g in gens:
                for _ in g:
                    pass
        else:
            while not done[0] and last[0] == "f":
                step(0)
                if BS > 1 and not done[1] and last[1] == "f":
                    step(1)
            for b in range(BS):
                nxt = b + 1 if b + 1 < BS else None
                while not done[b]:
                    step(b)
                    if nxt is not None and not done[nxt] and last[nxt] == "f":
                        step(nxt)


def _dw_rhs(xf8, j):
    """Depthwise moving AP: tap pair (j-3, j+1) as two gap-4 groups over a
    [128, 520] zero-padded fp8 tile (data at cols 4..516)."""
    base = xf8[:, 0:512]
    return APc(base.tensor, base.offset + 1 + j,
               [list(base.ap[0]), [4, 2], [1, 512]])


def _pw_rhs(dwout8, cpair):
    """Pointwise moving AP: blocks (2c, 2c+1) of the [128, 2048] fp8 dwout
    tile as the two groups."""
    base = dwout8[:, 0:512]
    return APc(base.tensor, base.offset + 1024 * cpair,
               [list(base.ap[0]), [512, 2], [1, 512]])


def _dump(nc, wd, b, name, tiles, rows=128):
    dbg = wd.get("_dbg")
    import os
    if dbg is None or b != int(os.environ.get("BASSDBG_B", "0")) or name not in dbg:
        return
    for i, t in enumerate(tiles):
        nc.sync.dma_start(wd["_dbg"][name][128 * i:128 * i + rows, :],
                          t[0:rows, :] if rows < 128 else t[:])


def _trace_batch(nc, tc, b, wd, xT_d, orix_d, xmask_d, out_d,
                 dw_lhsT, pw_lhsT, inwT, wv, owT, ffcT, peT, G, E, C_t,
                 inb_cols, outb_col, ffcb_col, db_cols, pb_cols,
                 xpool, f8pool, mpool, qkpool, epool, apool, opool,
                 pp, pa, mask_any, bias_any, pad_any):
    # ---------------- pos_emb + input load ----------------
    if pad_any:
        mrow = mpool.tile([1, T], I32, tag="mrow_i", name="mrow_i")
        nc.scalar.dma_start(mrow[:], orix_d[b:b + 1, :])
        mrow_f = mpool.tile([1, T], F32, tag="mrow_f", name="mrow_f")
        nc.vector.tensor_copy(mrow_f[:], mrow[:])
        nc.vector.tensor_scalar_min(mrow_f[:], mrow_f[:], 1.0)
        dbgd = wd.get("_dbg")
        import os as _os
        if dbgd is not None and b == int(_os.environ.get("BASSDBG_B", "0")):
            nc.sync.dma_start(dbgd["mbc"][1:2, :], mrow_f[:])
        m_bc = mpool.tile([128, T], F32, tag="m_bc", name="m_bc", bufs=1)
        nc.gpsimd.partition_broadcast(m_bc[:], mrow_f[:])
        _dump(nc, wd, b, "mbc", [m_bc])
    xin = [xpool.tile([128, T], F32, tag=f"xin{ct}", name=f"xin{ct}", bufs=1)
           for ct in range(CT)]
    for ct in range(CT):
        r = _rows(ct)
        if r < 128:
            nc.gpsimd.memset(xin[ct][96:128, :], 0.0)
        nc.scalar.dma_start(xin[ct][0:r, :], xT_d[b, 128 * ct:128 * ct + r, :])
    xcur = [xpool.tile([128, T], F32R, tag=f"x{ct}", name=f"x{ct}") for ct in range(CT)]
    if pad_any:
        for ct in range(CT):
            pem = mpool.tile([128, T], F32, tag="pem", name="pem", bufs=1)
            nc.vector.tensor_tensor(pem[:], peT[ct][:], m_bc[:], op=ALU.mult)
            nc.vector.tensor_tensor(xcur[ct][:], xin[ct][:], pem[:], op=ALU.add)
    else:
        for ct in range(CT):
            nc.vector.tensor_tensor(xcur[ct][:], xin[ct][:], peT[ct][:],
                                    op=ALU.add)
    nc.scalar.dma_start(xcur[3][116:117, :], wd["onesrow"][:])
    _dump(nc, wd, b, "pos", xcur)

    yield "f"
    # ---------------- 4x sepconv + residual ----------------
    for l in range(4):
        # fp8 conv input tiles, zero-padded borders (cols 0:4 and 516:520)
        xf8 = []
        for ct in range(CT):
            t = f8pool.tile([128, XW], FP8, tag=f"xf8_{ct}", name=f"xf8_{ct}")
            bord = APc(t[:, 0:1].tensor, t[:, 0:1].offset,
                       [list(t[:, 0:1].ap[0]), [516, 2], [1, 4]])
            nc.gpsimd.memset(bord, 0.0)
            nc.gpsimd.tensor_copy(t[:, 4:516], xcur[ct][:])
            xf8.append(t)
        dwout8 = f8pool.tile([128, 2048], FP8, tag="dwout8", name="dwout8")
        for blk in range(CT):
            pdw = pp.tile([128, T], F32, tag="ps", name="ps")
            first = True
            for half in range(2):
                for j in range(4):
                    nc.tensor.matmul(pdw[:], dw_lhsT(l, blk, half, j),
                                     _dw_rhs(xf8[blk], j),
                                     start=first, stop=(half == 1 and j == 3),
                                     perf_mode=DR, skip_group_check=True)
                    first = False
            if bias_any:
                nc.scalar.activation(dwout8[:, 512 * blk:512 * (blk + 1)],
                                     pdw[:], AF.Identity,
                                     bias=db_cols[l][:, blk:blk + 1])
            else:
                nc.vector.tensor_copy(dwout8[:, 512 * blk:512 * (blk + 1)],
                                      pdw[:])
            if blk == 1:
                yield "f"
        yield "f"
        xnext = [xpool.tile([128, T], F32R, tag=f"x{ot}", name=f"x{ot}") for ot in range(CT)]
        for ot in range(CT):
            ppw = pp.tile([128, T], F32, tag="ps", name="ps")
            first = True
            for half in range(2):
                for cpair in range(2):
                    nc.tensor.matmul(ppw[:], pw_lhsT(l, half, cpair, ot),
                                     _pw_rhs(dwout8, cpair),
                                     start=first,
                                     stop=(half == 1 and cpair == 1),
                                     perf_mode=DR, skip_group_check=True)
                    first = False
            if bias_any:
                nc.vector.scalar_tensor_tensor(xnext[ot][:], ppw[:],
                                               pb_cols[l][:, ot:ot + 1],
                                               xcur[ot][:],
                                               op0=ALU.add, op1=ALU.add)
            else:
                nc.vector.tensor_tensor(xnext[ot][:], ppw[:], xcur[ot][:],
                                        op=ALU.add)
        xcur = xnext
        if l == 0:
            _dump(nc, wd, b, "l1", xcur)
        if l == 3:
            _dump(nc, wd, b, "l4", xcur)
        yield "f"

    # ---------------- attention ----------------
    # q (p=0..4) and k (p=5..9) pair tiles, f32r
    qk = []
    for p in range(10):
        pq = pp.tile([128, T], F32, tag="ps", name="ps")
        for ct in range(CT):
            nc.tensor.matmul(pq[:], inwT[ct][:, 128 * p:128 * (p + 1)],
                             xcur[ct][:], start=(ct == 0), stop=(ct == CT - 1))
        qt = qkpool.tile([128, T], F32R, tag=f"qk{p}", name=f"qk{p}")
        if bias_any:
            nc.scalar.activation(qt[:], pq[:], AF.Identity,
                                 bias=inb_cols[:, p:p + 1])
        else:
            nc.scalar.activation(qt[:], pq[:], AF.Identity)
        qk.append(qt)
        if p == 0:
            _dump(nc, wd, b, "qk0", [qt])
        if p == 5:
            _dump(nc, wd, b, "qk5", [qt])
        if p % 3 == 2:
            yield "b"
    # v^T (+ ones column): per kt: [128, 512] + [128, 256] psum pieces
    vaug = []
    for kt in range(CT):
        pv0 = pp.tile([128, T], F32, tag="ps", name="ps")
        pv1 = pp.tile([128, VW - T], F32, tag="ps", name="ps")
        for ct in range(CT):
            nc.tensor.matmul(pv0[:], xcur[ct][:, 128 * kt:128 * (kt + 1)],
                             wv[ct][:, 0:512], start=(ct == 0), stop=(ct == CT - 1))
            nc.tensor.matmul(pv1[:], xcur[ct][:, 128 * kt:128 * (kt + 1)],
                             wv[ct][:, 512:VW], start=(ct == 0), stop=(ct == CT - 1))
        vt = qkpool.tile([128, HP], BF16, tag=f"vaug{kt}", name=f"vaug{kt}", bufs=2)
        if bias_any:
            nc.vector.tensor_tensor(vt[:, 0:512], pv0[:], C_t[:, 0:512], op=ALU.add)
            nc.vector.tensor_tensor(vt[:, 512:HP], pv1[:, 0:128], C_t[:, 512:HP],
                                    op=ALU.add)
        else:
            nc.scalar.activation(vt[:, 0:512], pv0[:], AF.Identity)
            nc.scalar.activation(vt[:, 512:HP], pv1[:, 0:128], AF.Identity)
        vaug.append(vt)
        if kt == 0:
            _dump(nc, wd, b, "vaug", [vt[:, 0:512]])
        if kt % 2 == 1:
            yield "b"
    keep = None
    if mask_any:
        keep = []
        for kt in range(CT):
            kc_u8 = mpool.tile([128, 1], U8, tag=f"kc8_{kt}", name=f"kc8_{kt}")
            nc.sync.dma_start(
                kc_u8[:],
                xmask_d[b, 128 * kt:128 * (kt + 1)].rearrange(
                    "(t one) -> t one", one=1))
            kc = mpool.tile([128, 1], F32, tag=f"kc{kt}", name=f"kc{kt}")
            nc.vector.tensor_copy(kc[:], kc_u8[:])
            nc.vector.tensor_scalar(kc[:], kc[:], -1.0, 1.0,
                                    op0=ALU.mult, op1=ALU.add)
            keep.append(kc)

    abuf = []
    for p in range(5):
        pat = pa.tile([128, T], F32, tag="pat", name="pat", bufs=2)
        for h in (2 * p, 2 * p + 1):
            s = 64 * (h % 2)
            expt = []
            for m in range(CT):
                ps_ = pp.tile([128, T], F32, tag="ps", name="ps")
                nc.tensor.matmul(ps_[:], qk[5 + p][s:s + 64, 128 * m:128 * (m + 1)],
                                 qk[p][s:s + 64, :], start=True, stop=True)
                et = epool.tile([128, T], BF16, tag="exp", name="exp", bufs=6)
                nc.scalar.activation(et[:], ps_[:], AF.Exp)
                if keep is not None:
                    nc.vector.tensor_scalar_mul(et[:], et[:], keep[m][:])
                expt.append(et)
            yield "b"
            for m in range(CT):
                nc.tensor.matmul(pat[s:s + 64, :],
                                 vaug[m][:, 128 * p + s:128 * p + s + 64],
                                 expt[m][:], start=(m == 0), stop=(m == CT - 1))
        ab = apool.tile([128, T], BF16, tag=f"abuf{p}", name=f"abuf{p}", bufs=1)
        nc.vector.tensor_copy(ab[:], pat[:])
        abuf.append(ab)
        if p == 0:
            _dump(nc, wd, b, "abuf", [ab])
        yield "b"
    pr = pp.tile([H, T], F32, tag="ps", name="ps")
    for p in range(5):
        nc.tensor.matmul(pr[:], G[p][:], abuf[p][:],
                         start=(p == 0), stop=(p == 4))
    rrec = apool.tile([H, T], BF16, tag="rrec", name="rrec", bufs=1)
    with nc.allow_low_precision(reason="softmax recip; normalized weights"):
        nc.vector.reciprocal(rrec[:], pr[:])
    yield "b"
    anorm = []
    for p in range(5):
        pbc = pp.tile([128, T], F32, tag="ps", name="ps")
        nc.tensor.matmul(pbc[:], E[p][:], rrec[:], start=True, stop=True)
        an = apool.tile([128, T], BF16, tag=f"anorm{p}", name=f"anorm{p}", bufs=1)
        nc.vector.tensor_tensor(an[:], abuf[p][:], pbc[:], op=ALU.mult)
        anorm.append(an)
    _dump(nc, wd, b, "anorm", [anorm[0]])
    # out-proj + residual
    x2 = [xpool.tile([128, T], F32R, tag=f"x{ot}", name=f"x{ot}") for ot in range(CT)]
    for ot in range(CT):
        po = pp.tile([128, T], F32, tag="ps", name="ps")
        for p in range(5):
            nc.tensor.matmul(po[:], owT[p][:, 128 * ot:128 * (ot + 1)],
                             anorm[p][:], start=(p == 0), stop=(p == 4))
        if bias_any:
            nc.vector.scalar_tensor_tensor(x2[ot][:], po[:],
                                           outb_col[:, ot:ot + 1], xcur[ot][:],
                                           op0=ALU.add, op1=ALU.add)
        else:
            nc.vector.tensor_tensor(x2[ot][:], po[:], xcur[ot][:], op=ALU.add)
    _dump(nc, wd, b, "x2", x2)
    yield "b"

    # ---------------- ffc + residual + store ----------------
    for ot in range(CT):
        pf = pp.tile([128, T], F32, tag="ps", name="ps")
        for ct in range(CT):
            nc.tensor.matmul(pf[:], ffcT[ct][:, 128 * ot:128 * (ot + 1)],
                             x2[ct][:], start=(ct == 0), stop=(ct == CT - 1))
        ott = opool.tile([128, T], F32, tag=f"out{ot}", name=f"out{ot}", bufs=1)
        if bias_any:
            nc.vector.scalar_tensor_tensor(ott[:], pf[:],
                                           ffcb_col[:, ot:ot + 1], x2[ot][:],
                                           op0=ALU.add, op1=ALU.add)
        else:
            nc.vector.tensor_tensor(ott[:], pf[:], x2[ot][:], op=ALU.add)
        r = _rows(ot)
        nc.sync.dma_start(out_d[b, 128 * ot:128 * ot + r, :], ott[0:r, :])


_CACHE = {}


def _get_program(consts, mask_any, bias_any, pad_any):
    key = (mask_any, bias_any, pad_any)
    if key not in _CACHE:
        _CACHE[key] = trace_program(consts, mask_any, bias_any, pad_any)
    return _CACHE[key]


def kernel(ori_x, x, x_mask,
           dw1, db1, pw1, pb1, dw2, db2, pw2, pb2,
           dw3, db3, pw3, pb3, dw4, db4, pw4, pb4,
           in_w, in_b, out_w, out_b, ffc_w, ffc_b, _results=None):
    ori_x = np.asarray(ori_x)
    x = np.asarray(x, dtype=np.float32)
    x_mask = np.asarray(x_mask)
    consts = build_host_consts(
        [np.asarray(d, np.float32) for d in (dw1, dw2, dw3, dw4)],
        [np.asarray(p, np.float32) for p in (pw1, pw2, pw3, pw4)],
        [np.asarray(d, np.float32) for d in (db1, db2, db3, db4)],
        [np.asarray(p, np.float32) for p in (pb1, pb2, pb3, pb4)],
        np.asarray(in_w, np.float32), np.asarray(in_b, np.float32),
        np.asarray(out_w, np.float32), np.asarray(out_b, np.float32),
        np.asarray(ffc_w, np.float32), np.asarray(ffc_b, np.float32))
    bias_any = any(np.any(np.asarray(v)) for v in
                   (db1, db2, db3, db4, pb1, pb2, pb3, pb4, in_b, out_b, ffc_b))
    mask_any = bool(np.asarray(x_mask).any())
    pad_any = bool((np.asarray(ori_x) == 0).any())
    nc = _get_program(consts, mask_any, bias_any, pad_any)

    xT = np.ascontiguousarray(x.transpose(0, 2, 1))       # [B, D, T]
    ori32 = ori_x.astype(np.int32)
    mask8 = x_mask.astype(np.uint8)
    in_maps = []
    for c in range(NC_):
        sl = slice(BS * c, BS * (c + 1))
        m = {"xT": xT[sl], "orix": ori32[sl], "xmask": mask8[sl]}
        m.update({k: v for k, v in consts.items() if not k.startswith("_")})
        in_maps.append(m)
    res = run_bass_kernel_spmd(nc, in_maps, list(range(NC_)))
    if _results is not None:
        _results.append(res)
    outT = np.concatenate([res.results[c]["out"] for c in range(NC_)], axis=0)
    return np.ascontiguousarray(outT.transpose(0, 2, 1)).astype(np.float32)


# revision 19
# speedup vs baseline: 1.0137x; 1.0137x over previous
"""Trainium2 Bass kernel for nn_Encoder_78649441124984.

Encoder: pos_emb + 4x(sepconv+res) + MHA(+res) + ffc(+res).
Sharding: data-parallel over batch, 8 cores x 4 batch elements, all
parameters replicated; no collectives.

On-device layout: activations kept transposed as [feature, time] tiles
([128, 512] SBUF tiles, feature on partitions).

Speed structure (v1):
 - sepconv (depthwise + pointwise) runs in fp8e4m3 with DoubleRow perf
   mode (2 contraction groups per pass at 0.5 cycles/row).  Weight
   quantization error is cancelled by a hi/lo split: W ~= fp8(W) +
   fp8(W - fp8(W)), two DoubleRow passes.  Depthwise pairs taps
   (s, s+4) as the two groups of one DoubleRow matmul over a
   zero-padded fp8 input tile.
 - everything else (qkv, scores, av, out/ffc proj) runs in
   float32r: at moving size >= 256 the PE runs f32r at bf16 speed,
   so this is free accuracy.  vaug/expt/anorm/owT run bf16 (their
   quantization is harmless); the residual stream stays f32r.
"""
import sys

sys.path.insert(0, "/opt/trn_rl_repo")

import numpy as np
import ml_dtypes

import concourse.bass as bass
import concourse.mybir as mybir
import concourse.tile as tile
from concourse import bacc
from concourse.ap import AP as APc
from concourse.bass_utils import run_bass_kernel_spmd

F32 = mybir.dt.float32
F32R = mybir.dt.float32r
BF16 = mybir.dt.bfloat16
FP8 = mybir.dt.float8e4
I32 = mybir.dt.int32
U8 = mybir.dt.uint8
AF = mybir.ActivationFunctionType
ALU = mybir.AluOpType
DR = mybir.MatmulPerfMode.DoubleRow
FP8NP = ml_dtypes.float8_e4m3
BF16NP = ml_dtypes.bfloat16

D = 500
H = 10
HD = 50
B, T = 32, 512
K = 7
NC_ = 8
BS = B // NC_          # batch shard per core
DP = 512               # padded feature dim
CT = 4                 # feature tiles (4 x 128 = 512 >= 500)
HP = 640               # padded head dim total (10 heads x 64 slots)
VW = 768               # v-proj rhs width (640 padded to 768 so the
                       # second psum piece has moving size 256)
XW = 520               # padded conv input tile width (4 + 512 + 4)


def _rows(ct):
    return min(128, D - 128 * ct)


def _head_col(h):
    return 128 * (h // 2) + 64 * (h % 2)


def _q8(a):
    return a.astype(FP8NP).astype(np.float32)


def build_host_consts(dw, pw, db, pb, in_w, in_b, out_w, out_b, ffc_w, ffc_b):
    """Pack all weights into device layouts. dw: [4][D,1,K], pw: [4][D,D]."""
    c = {}
    # ---------------- fp8 wall: depthwise diag pairs + pointwise ----------
    # depthwise: per (layer, block): 8 DoubleRow lhsT of [128, 2, 128]:
    # halves (hi, lo) x tap-pairs j=0..3 with taps (j-3, j+1); tap 4 = 0.
    # pointwise: per layer: halves (hi, lo) x ct-pairs c=0,1 of
    # [128, 2, 512]: group g holds pwT rows of ct=2c+g.
    w8_secs = []
    for l in range(4):
        dwf = dw[l][:, 0, :]                      # [D, K]
        dwp = np.zeros((DP, K + 2), np.float32)   # taps -3..3 plus zero tap 4
        dwp[:D, :K] = dwf
        hi = _q8(dwp)
        lo = dwp - hi                             # fp8 cast below
        diag = np.zeros((2, CT, 4, 128, 2, 128), np.float32)
        for half, w in ((0, hi), (1, lo)):
            for blk in range(CT):
                for j in range(4):
                    for g, tap in ((0, j), (1, j + 4)):
                        np.fill_diagonal(diag[half, blk, j, :, g, :],
                                         w[128 * blk:128 * blk + 128, tap])
        w8_secs.append((f"diag{l}", diag.reshape(2, CT, 4, 128, 256)
                        .transpose(3, 1, 0, 2, 4).reshape(128, -1)))
        # cols layout: blk-major, then half, then j, then [2x128]
        pwT = np.zeros((DP, DP), np.float32)
        pwT[:D, :D] = pw[l].T
        pwh = _q8(pwT)
        pwl = pwT - pwh
        pk = np.zeros((2, 2, 128, 2, DP), np.float32)
        for half, w in ((0, pwh), (1, pwl)):
            for cpair in range(2):
                for g in range(2):
                    ct = 2 * cpair + g
                    pk[half, cpair, :, g, :] = w[128 * ct:128 * ct + 128, :]
        # layout: half-major, then cpair, then [128, 2, 512] -> [128, 2048]
        w8_secs.append((f"pw{l}", pk.transpose(2, 0, 1, 3, 4).reshape(128, -1)))
    offs8 = {}
    w = 0
    parts = []
    for name, arr in w8_secs:
        offs8[name] = w
        w += arr.shape[1]
        parts.append(arr)
    c["wall8"] = np.concatenate(parts, 1).astype(FP8NP)
    c["_offs8"] = offs8

    # ---------------- f32r wall: qkv in-proj, v-proj, G ------------------
    scale = HD ** -0.5
    inwT = np.zeros((DP, 2 * HP), np.float32)
    inb_cols = np.zeros((128, 10), np.float32)
    for h in range(H):
        base = _head_col(h)
        qrows = slice(100 * (h // 2) + 50 * (h % 2),
                      100 * (h // 2) + 50 * (h % 2) + 50)
        inwT[:D, base:base + 50] = in_w[qrows, :].T * scale
        inb_cols[base % 128:base % 128 + 50, h // 2] = in_b[qrows] * scale
        krows = slice(500 + qrows.start, 500 + qrows.stop)
        inwT[:D, HP + base:HP + base + 50] = in_w[krows, :].T
        inb_cols[base % 128:base % 128 + 50, 5 + h // 2] = in_b[krows]
    wv = np.zeros((DP, VW), np.float32)
    crow = np.zeros((1, HP), np.float32)
    for h in range(H):
        base = _head_col(h)
        vrows = slice(1000 + 50 * h, 1000 + 50 * h + 50)
        wv[:D, base:base + 50] = in_w[vrows, :].T
        wv[D, base + 50] = 1.0    # ones column via the stream's 1.0 row
        crow[0, base:base + 50] = in_b[vrows]
        crow[0, base + 50] = 1.0
    G = np.zeros((5 * 128, H), np.float32)
    E = np.zeros((5 * H, 128), np.float32)
    for p in range(5):
        G[128 * p + 50, 2 * p] = 1.0
        G[128 * p + 114, 2 * p + 1] = 1.0
        E[H * p + 2 * p, 0:50] = 1.0
        E[H * p + 2 * p + 1, 64:114] = 1.0
    f32_secs = [("inwT", _rt(inwT)), ("wv", _rt(wv)),
                ("ffcT", _rt(np.pad(ffc_w.T, ((0, DP - D), (0, DP - D)))))]
    offsf = {}
    w = 0
    parts = []
    for name, arr in f32_secs:
        offsf[name] = w
        w += arr.shape[1]
        parts.append(arr)
    c["wallf"] = np.concatenate(parts, 1).astype(np.float32)
    c["_offsf"] = offsf

    # ---------------- bf16 wall: out-proj + ffc --------------------------
    owT = np.zeros((HP, DP), np.float32)
    for h in range(H):
        base = _head_col(h)
        owT[base:base + 50, :D] = out_w[:, 50 * h:50 * h + 50].T
    b16_secs = [("owT", _rt(owT)), ("G", _rt(G))]
    offsb = {}
    w = 0
    parts = []
    for name, arr in b16_secs:
        offsb[name] = w
        w += arr.shape[1]
        parts.append(arr)
    c["wallb"] = np.concatenate(parts, 1).astype(BF16NP)
    c["_offsb"] = offsb

    # ---------------- small f32 tensors ----------------------------------
    sm = np.concatenate(
        [inb_cols,
         np.pad(out_b, (0, DP - D)).reshape(CT, 128).T,
         np.pad(ffc_b, (0, DP - D)).reshape(CT, 128).T,
         np.concatenate([np.pad(db[l], (0, DP - D)).reshape(CT, 128).T
                         for l in range(4)], 1),
         np.concatenate([np.pad(pb[l], (0, DP - D)).reshape(CT, 128).T
                         for l in range(4)], 1)], 1).astype(np.float32)
    c["smallf"] = sm
    c["E_all"] = np.ascontiguousarray(
        np.concatenate([E[10 * p:10 * (p + 1), :] for p in range(5)], 1)
    ).astype(BF16NP)
    c["crow"] = crow.astype(np.float32)
    c["onesrow"] = np.ones((1, T), np.float32)
    half = D // 2
    inv = np.exp(np.arange(half, dtype=np.float64) * (-np.log(10000.0) / (half - 1)))
    pos = np.arange(1, T + 1, dtype=np.float64)
    ang = pos[None, :] * inv[:, None]
    peT = np.zeros((DP, T), np.float32)
    peT[:half, :] = np.sin(ang)
    peT[half:D, :] = np.cos(ang)
    c["peTp"] = _rt(peT).astype(BF16NP)
    return c


def _rt(a):
    """Repack row-tiled [n*128, C] -> [128, n*C] (tile ct at cols ct*C)."""
    n = a.shape[0] // 128
    return a.reshape(n, 128, a.shape[1]).transpose(1, 0, 2).reshape(128, -1)


def trace_program(consts, mask_any, bias_any, pad_any):
    """Build the SPMD Bass program (same for every core)."""
    nc = bacc.Bacc("TRN2", target_bir_lowering=False, debug=False,
                   num_devices=NC_)

    import os
    dbg = os.environ.get("BASSDBG") == "1"
    xT_d = nc.dram_tensor("xT", [BS, D, T], F32, kind="ExternalInput")
    orix_d = nc.dram_tensor("orix", [BS, T], I32, kind="ExternalInput")
    xmask_d = nc.dram_tensor("xmask", [BS, T], U8, kind="ExternalInput")
    out_d = nc.dram_tensor("out", [BS, D, T], F32, kind="ExternalOutput")
    dbg_d = None
    if dbg:
        dbg_d = {}
        for n in ("pos", "l1", "l4", "qk0", "qk5", "x2"):
            dbg_d[n] = nc.dram_tensor(f"dbg_{n}", [512, T], F32R, kind="ExternalOutput")
        for n in ("vaug", "abuf", "anorm", "mbc"):
            dbg_d[n] = nc.dram_tensor(f"dbg_{n}", [512, T], BF16, kind="ExternalOutput")

    wd = {"_offs8": consts["_offs8"], "_offsf": consts["_offsf"],
          "_offsb": consts["_offsb"]}
    dts = {"wall8": FP8, "wallf": F32R, "wallb": BF16, "smallf": F32,
           "E_all": BF16, "crow": F32, "peTp": BF16, "onesrow": F32R}
    for name, arr in consts.items():
        if name.startswith("_"):
            continue
        wd[name] = nc.dram_tensor(name, list(arr.shape), dts[name],
                                  kind="ExternalInput")

    with tile.TileContext(nc, num_cores=NC_) as tc:
        wd["_dbg"] = dbg_d
        _trace_body(nc, tc, wd, xT_d, orix_d, xmask_d, out_d,
                    mask_any, bias_any, pad_any)
    nc.finalize()
    return nc


def _trace_body(nc, tc, wd, xT_d, orix_d, xmask_d, out_d,
                mask_any, bias_any, pad_any):
    from contextlib import ExitStack
    ctx = ExitStack()
    with ctx:
        wpool = ctx.enter_context(tc.tile_pool(name="w", bufs=1))
        offs8 = wd["_offs8"]
        offsf = wd["_offsf"]
        offsb = wd["_offsb"]
        W8 = wd["wall8"].shape[1]
        WF = wd["wallf"].shape[1]
        WB = wd["wallb"].shape[1]
        wall8 = wpool.tile([128, W8], FP8, tag="wall8", name="wall8")
        wallf = wpool.tile([128, WF], F32R, tag="wallf", name="wallf")
        wallb = wpool.tile([128, WB], BF16, tag="wallb", name="wallb")
        # urgent small constants on the ACT ring
        peTp = wpool.tile([128, CT * T], BF16, tag="peTp", name="peTp")
        nc.scalar.dma_start(peTp[:], wd["peTp"][:])
        smallf = wpool.tile([128, 50], F32, tag="smallf", name="smallf")
        nc.scalar.dma_start(smallf[:], wd["smallf"][:])
        E_t = wpool.tile([H, 5 * 128], BF16, tag="E_t", name="E_t")
        nc.scalar.dma_start(E_t[:], wd["E_all"][:])
        crow_t = None
        if bias_any:
            crow_t = wpool.tile([1, HP], F32, tag="crow", name="crow")
            nc.scalar.dma_start(crow_t[:], wd["crow"][:])
        # walls on the SP ring, section-by-section in first-use order
        def sec_dmas(tile_t, dram, offd, order, width):
            sw = {}
            for s in offd:
                nxt = [offd[t] for t in offd if offd[t] > offd[s]]
                sw[s] = (min(nxt) if nxt else width) - offd[s]
            for s in order:
                nc.sync.dma_start(tile_t[:, offd[s]:offd[s] + sw[s]],
                                  dram[:, offd[s]:offd[s] + sw[s]])
        o8 = []
        for l in range(4):
            o8 += [f"diag{l}", f"pw{l}"]
        sec_dmas(wall8, wd["wall8"], offs8, o8, W8)
        sec_dmas(wallf, wd["wallf"], offsf, ["inwT", "wv", "ffcT"], WF)
        sec_dmas(wallb, wd["wallb"], offsb, ["owT", "G"], WB)
        C_t = None
        if bias_any:
            C_t = wpool.tile([128, HP], F32, tag="C", name="C")
            nc.gpsimd.partition_broadcast(C_t[:], crow_t[:])

        # weight-slice helpers -------------------------------------------
        def dw_lhsT(l, blk, half, j):
            off = offs8[f"diag{l}"] + blk * 2048 + half * 1024 + j * 256
            base = wall8[:, off:off + 256]
            return APc(base.tensor, base.offset,
                       [list(base.ap[0]), [128, 2], [1, 128]])

        def pw_lhsT(l, half, cpair, ot):
            off = offs8[f"pw{l}"] + half * 2048 + cpair * 1024 + 128 * ot
            base = wall8[:, off:off + 1]
            return APc(base.tensor, base.offset,
                       [list(base.ap[0]), [512, 2], [1, 128]])

        inwT = [wallf[:, offsf["inwT"] + 2 * HP * ct:
                       offsf["inwT"] + 2 * HP * (ct + 1)] for ct in range(CT)]
        wv = [wallf[:, offsf["wv"] + VW * ct:offsf["wv"] + VW * (ct + 1)]
              for ct in range(CT)]
        G = [wallb[:, offsb["G"] + H * p:offsb["G"] + H * (p + 1)]
             for p in range(5)]
        owT = [wallb[:, offsb["owT"] + DP * p:offsb["owT"] + DP * (p + 1)]
               for p in range(5)]
        ffcT = [wallf[:, offsf["ffcT"] + DP * ct:offsf["ffcT"] + DP * (ct + 1)]
                for ct in range(CT)]
        E = [E_t[:, 128 * p:128 * (p + 1)] for p in range(5)]
        peT = [peTp[:, T * ct:T * (ct + 1)] for ct in range(CT)]
        inb_cols = smallf[:, 0:10]
        outb_col = smallf[:, 10:14]
        ffcb_col = smallf[:, 14:18]
        db_cols = [smallf[:, 18 + CT * l:18 + CT * (l + 1)] for l in range(4)]
        pb_cols = [smallf[:, 34 + CT * l:34 + CT * (l + 1)] for l in range(4)]

        # ---- per-batch-element pools ----
        xpool = ctx.enter_context(tc.tile_pool(name="x", bufs=3))
        f8pool = ctx.enter_context(tc.tile_pool(name="f8", bufs=2))
        mpool = ctx.enter_context(tc.tile_pool(name="m", bufs=2))
        qkpool = ctx.enter_context(tc.tile_pool(name="qk", bufs=1))
        epool = ctx.enter_context(tc.tile_pool(name="e", bufs=2))
        apool = ctx.enter_context(tc.tile_pool(name="a", bufs=2))
        opool = ctx.enter_context(tc.tile_pool(name="o", bufs=2))
        pp = ctx.enter_context(tc.tile_pool(name="pp", bufs=6, space="PSUM"))
        pa = ctx.enter_context(tc.tile_pool(name="pa", bufs=1, space="PSUM"))

        gens = [
            _trace_batch(nc, tc, b, wd, xT_d, orix_d, xmask_d, out_d,
                         dw_lhsT, pw_lhsT, inwT, wv, owT, ffcT, peT, G, E, C_t,
                         inb_cols, outb_col, ffcb_col, db_cols, pb_cols,
                         xpool, f8pool, mpool, qkpool, epool, apool, opool,
                         pp, pa, mask_any, bias_any, pad_any)
            for b in range(BS)
        ]
        done = [False] * BS
        last = ["f"] * BS

        def step(i):
            try:
                last[i] = next(gens[i])
            except StopIteration:
                done[i] = True

        import os
        if os.environ.get("NOILV") == "1":
            for g in gens:
                for _ in g:
                    pass
        else:
            while not done[0] and last[0] == "f":
                step(0)
                if BS > 1 and not done[1] and last[1] == "f":
                    step(1)
            for b in range(BS):
                nxt = b + 1 if b + 1 < BS else None
                while not done[b]:
                    step(b)
                    if nxt is not None and not done[nxt] and last[nxt] == "f":
                        step(nxt)


def _dw_rhs(xf8, j):
    """Depthwise moving AP: tap pair (j-3, j+1) as two gap-4 groups over a
    [128, 520] zero-padded fp8 tile (data at cols 4..516)."""
    base = xf8[:, 0:512]
    return APc(base.tensor, base.offset + 1 + j,
               [list(base.ap[0]), [4, 2], [1, 512]])


def _pw_rhs(dwout8, cpair):
    """Pointwise moving AP: blocks (2c, 2c+1) of the [128, 2048] fp8 dwout
    tile as the two groups."""
    base = dwout8[:, 0:512]
    return APc(base.tensor, base.offset + 1024 * cpair,
               [list(base.ap[0]), [512, 2], [1, 512]])


def _dump(nc, wd, b, name, tiles, rows=128):
    dbg = wd.get("_dbg")
    import os
    if dbg is None or b != int(os.environ.get("BASSDBG_B", "0")) or name not in dbg:
        return
    for i, t in enumerate(tiles):
        nc.sync.dma_start(wd["_dbg"][name][128 * i:128 * i + rows, :],
                          t[0:rows, :] if rows < 128 else t[:])


def _trace_batch(nc, tc, b, wd, xT_d, orix_d, xmask_d, out_d,
                 dw_lhsT, pw_lhsT, inwT, wv, owT, ffcT, peT, G, E, C_t,
                 inb_cols, outb_col, ffcb_col, db_cols, pb_cols,
                 xpool, f8pool, mpool, qkpool, epool, apool, opool,
                 pp, pa, mask_any, bias_any, pad_any):
    # ---------------- pos_emb + input load ----------------
    if pad_any:
        mrow = mpool.tile([1, T], I32, tag="mrow_i", name="mrow_i")
        nc.scalar.dma_start(mrow[:], orix_d[b:b + 1, :])
        mrow_f = mpool.tile([1, T], F32, tag="mrow_f", name="mrow_f")
        nc.vector.tensor_copy(mrow_f[:], mrow[:])
        nc.vector.tensor_scalar_min(mrow_f[:], mrow_f[:], 1.0)
        dbgd = wd.get("_dbg")
        import os as _os
        if dbgd is not None and b == int(_os.environ.get("BASSDBG_B", "0")):
            nc.sync.dma_start(dbgd["mbc"][1:2, :], mrow_f[:])
        m_bc = mpool.tile([128, T], F32, tag="m_bc", name="m_bc", bufs=1)
        nc.gpsimd.partition_broadcast(m_bc[:], mrow_f[:])
        _dump(nc, wd, b, "mbc", [m_bc])
    xin = [xpool.tile([128, T], F32, tag=f"xin{ct}", name=f"xin{ct}", bufs=1)
           for ct in range(CT)]
    for ct in range(CT):
        r = _rows(ct)
        if r < 128:
            nc.gpsimd.memset(xin[ct][96:128, :], 0.0)
        nc.scalar.dma_start(xin[ct][0:r, :], xT_d[b, 128 * ct:128 * ct + r, :])
    xcur = [xpool.tile([128, T], F32R, tag=f"x{ct}", name=f"x{ct}") for ct in range(CT)]
    if pad_any:
        for ct in range(CT):
            pem = mpool.tile([128, T], F32, tag="pem", name="pem", bufs=1)
            nc.vector.tensor_tensor(pem[:], peT[ct][:], m_bc[:], op=ALU.mult)
            nc.vector.tensor_tensor(xcur[ct][:], xin[ct][:], pem[:], op=ALU.add)
    else:
        for ct in range(CT):
            nc.vector.tensor_tensor(xcur[ct][:], xin[ct][:], peT[ct][:],
                                    op=ALU.add)
    nc.scalar.dma_start(xcur[3][116:117, :], wd["onesrow"][:])
    _dump(nc, wd, b, "pos", xcur)

    yield "f"
    # ---------------- 4x sepconv + residual ----------------
    for l in range(4):
        # fp8 conv input tiles, zero-padded borders (cols 0:4 and 516:520)
        xf8 = []
        for ct in range(CT):
            t = f8pool.tile([128, XW], FP8, tag=f"xf8_{ct}", name=f"xf8_{ct}")
            bord = APc(t[:, 0:1].tensor, t[:, 0:1].offset,
                       [list(t[:, 0:1].ap[0]), [516, 2], [1, 4]])
            nc.gpsimd.memset(bord, 0.0)
            nc.gpsimd.tensor_copy(t[:, 4:516], xcur[ct][:])
            xf8.append(t)
        dwout8 = f8pool.tile([128, 2048], FP8, tag="dwout8", name="dwout8")
        for blk in range(CT):
            pdw = pp.tile([128, T], F32, tag="ps", name="ps")
            first = True
            for half in range(2):
                for j in range(4):
                    nc.tensor.matmul(pdw[:], dw_lhsT(l, blk, half, j),
                                     _dw_rhs(xf8[blk], j),
                                     start=first, stop=(half == 1 and j == 3),
                                     perf_mode=DR, skip_group_check=True)
                    first = False
            if bias_any:
                nc.scalar.activation(dwout8[:, 512 * blk:512 * (blk + 1)],
                                     pdw[:], AF.Identity,
                                     bias=db_cols[l][:, blk:blk + 1])
            else:
                nc.scalar.activation(dwout8[:, 512 * blk:512 * (blk + 1)],
                                     pdw[:], AF.Identity)
            if blk == 1:
                yield "f"
        yield "f"
        xnext = [xpool.tile([128, T], F32R, tag=f"x{ot}", name=f"x{ot}") for ot in range(CT)]
        for ot in range(CT):
            ppw = pp.tile([128, T], F32, tag="ps", name="ps")
            first = True
            for half in range(2):
                for cpair in range(2):
                    nc.tensor.matmul(ppw[:], pw_lhsT(l, half, cpair, ot),
                                     _pw_rhs(dwout8, cpair),
                                     start=first,
                                     stop=(half == 1 and cpair == 1),
                                     perf_mode=DR, skip_group_check=True)
                    first = False
            if bias_any:
                nc.vector.scalar_tensor_tensor(xnext[ot][:], ppw[:],
                                               pb_cols[l][:, ot:ot + 1],
                                               xcur[ot][:],
                                               op0=ALU.add, op1=ALU.add)
            else:
                nc.vector.tensor_tensor(xnext[ot][:], ppw[:], xcur[ot][:],
                                        op=ALU.add)
        xcur = xnext
        if l == 0:
            _dump(nc, wd, b, "l1", xcur)
        if l == 3:
            _dump(nc, wd, b, "l4", xcur)
        yield "f"

    # ---------------- attention ----------------
    # q (p=0..4) and k (p=5..9) pair tiles, f32r
    qk = []
    for p in range(10):
        pq = pp.tile([128, T], F32, tag="ps", name="ps")
        for ct in range(CT):
            nc.tensor.matmul(pq[:], inwT[ct][:, 128 * p:128 * (p + 1)],
                             xcur[ct][:], start=(ct == 0), stop=(ct == CT - 1))
        qt = qkpool.tile([128, T], F32R, tag=f"qk{p}", name=f"qk{p}")
        if bias_any:
            nc.scalar.activation(qt[:], pq[:], AF.Identity,
                                 bias=inb_cols[:, p:p + 1])
        else:
            nc.scalar.activation(qt[:], pq[:], AF.Identity)
        qk.append(qt)
        if p == 0:
            _dump(nc, wd, b, "qk0", [qt])
        if p == 5:
            _dump(nc, wd, b, "qk5", [qt])
        if p % 3 == 2:
            yield "b"
    # v^T (+ ones column): per kt: [128, 512] + [128, 256] psum pieces
    vaug = []
    for kt in range(CT):
        pv0 = pp.tile([128, T], F32, tag="ps", name="ps")
        pv1 = pp.tile([128, VW - T], F32, tag="ps", name="ps")
        for ct in range(CT):
            nc.tensor.matmul(pv0[:], xcur[ct][:, 128 * kt:128 * (kt + 1)],
                             wv[ct][:, 0:512], start=(ct == 0), stop=(ct == CT - 1))
            nc.tensor.matmul(pv1[:], xcur[ct][:, 128 * kt:128 * (kt + 1)],
                             wv[ct][:, 512:VW], start=(ct == 0), stop=(ct == CT - 1))
        vt = qkpool.tile([128, HP], BF16, tag=f"vaug{kt}", name=f"vaug{kt}", bufs=2)
        if bias_any:
            nc.vector.tensor_tensor(vt[:, 0:512], pv0[:], C_t[:, 0:512], op=ALU.add)
            nc.vector.tensor_tensor(vt[:, 512:HP], pv1[:, 0:128], C_t[:, 512:HP],
                                    op=ALU.add)
        else:
            nc.scalar.activation(vt[:, 0:512], pv0[:], AF.Identity)
            nc.scalar.activation(vt[:, 512:HP], pv1[:, 0:128], AF.Identity)
        vaug.append(vt)
        if kt == 0:
            _dump(nc, wd, b, "vaug", [vt[:, 0:512]])
        if kt % 2 == 1:
            yield "b"
    keep = None
    if mask_any:
        keep = []
        for kt in range(CT):
            kc_u8 = mpool.tile([128, 1], U8, tag=f"kc8_{kt}", name=f"kc8_{kt}")
            nc.sync.dma_start(
                kc_u8[:],
                xmask_d[b, 128 * kt:128 * (kt + 1)].rearrange(
                    "(t one) -> t one", one=1))
            kc = mpool.tile([128, 1], F32, tag=f"kc{kt}", name=f"kc{kt}")
            nc.vector.tensor_copy(kc[:], kc_u8[:])
            nc.vector.tensor_scalar(kc[:], kc[:], -1.0, 1.0,
                                    op0=ALU.mult, op1=ALU.add)
            keep.append(kc)

    abuf = []
    for p in range(5):
        pat = pa.tile([128, T], F32, tag="pat", name="pat", bufs=2)
        for h in (2 * p, 2 * p + 1):
            s = 64 * (h % 2)
            expt = []
            for m in range(CT):
                ps_ = pp.tile([128, T], F32, tag="ps", name="ps")
                nc.tensor.matmul(ps_[:], qk[5 + p][s:s + 64, 128 * m:128 * (m + 1)],
                                 qk[p][s:s + 64, :], start=True, stop=True)
                et = epool.tile([128, T], BF16, tag="exp", name="exp", bufs=6)
                nc.scalar.activation(et[:], ps_[:], AF.Exp)
                if keep is not None:
                    nc.vector.tensor_scalar_mul(et[:], et[:], keep[m][:])
                expt.append(et)
            yield "b"
            for m in range(CT):
                nc.tensor.matmul(pat[s:s + 64, :],
                                 vaug[m][:, 128 * p + s:128 * p + s + 64],
                                 expt[m][:], start=(m == 0), stop=(m == CT - 1))
        ab = apool.tile([128, T], BF16, tag=f"abuf{p}", name=f"abuf{p}", bufs=1)
        nc.vector.tensor_copy(ab[:], pat[:])
        abuf.append(ab)
        if p == 0:
            _dump(nc, wd, b, "abuf", [ab])
        yield "b"
    pr = pp.tile([H, T], F32, tag="ps", name="ps")
    for p in range(5):
        nc.tensor.matmul(pr[:], G[p][:], abuf[p][:],
                         start=(p == 0), stop=(p == 4))
    rrec = apool.tile([H, T], BF16, tag="rrec", name="rrec", bufs=1)
    with nc.allow_low_precision(reason="softmax recip; normalized weights"):
        nc.vector.reciprocal(rrec[:], pr[:])
    yield "b"
    anorm = []
    for p in range(5):
        pbc = pp.tile([128, T], F32, tag="ps", name="ps")
        nc.tensor.matmul(pbc[:], E[p][:], rrec[:], start=True, stop=True)
        an = apool.tile([128, T], BF16, tag=f"anorm{p}", name=f"anorm{p}", bufs=1)
        nc.vector.tensor_tensor(an[:], abuf[p][:], pbc[:], op=ALU.mult)
        anorm.append(an)
    _dump(nc, wd, b, "anorm", [anorm[0]])
    # out-proj + residual
    x2 = [xpool.tile([128, T], F32R, tag=f"x{ot}", name=f"x{ot}") for ot in range(CT)]
    for ot in range(CT):
        po = pp.tile([128, T], F32, tag="ps", name="ps")
        for p in range(5):
            nc.tensor.matmul(po[:], owT[p][:, 128 * ot:128 * (ot + 1)],
                             anorm[p][:], start=(p == 0), stop=(p == 4))
        if bias_any:
            nc.vector.scalar_tensor_tensor(x2[ot][:], po[:],
                                           outb_col[:, ot:ot + 1], xcur[ot][:],
                                           op0=ALU.add, op1=ALU.add)
        else:
            nc.vector.tensor_tensor(x2[ot][:], po[:], xcur[ot][:], op=ALU.add)
    _dump(nc, wd, b, "x2", x2)
    yield "b"

    # ---------------- ffc + residual + store ----------------
    for ot in range(CT):
        pf = pp.tile([128, T], F32, tag="ps", name="ps")
        for ct in range(CT):
            nc.tensor.matmul(pf[:], ffcT[ct][:, 128 * ot:128 * (ot + 1)],
                             x2[ct][:], start=(ct == 0), stop=(ct == CT - 1))
        ott = opool.tile([128, T], F32, tag=f"out{ot}", name=f"out{ot}", bufs=1)
        if bias_any:
            nc.vector.scalar_tensor_tensor(ott[:], pf[:],
                                           ffcb_col[:, ot:ot + 1], x2[ot][:],
                                           op0=ALU.add, op1=ALU.add)
        else:
            nc.vector.tensor_tensor(ott[:], pf[:], x2[ot][:], op=ALU.add)
        r = _rows(ot)
        nc.sync.dma_start(out_d[b, 128 * ot:128 * ot + r, :], ott[0:r, :])


_CACHE = {}


def _get_program(consts, mask_any, bias_any, pad_any):
    key = (mask_any, bias_any, pad_any)
    if key not in _CACHE:
        _CACHE[key] = trace_program(consts, mask_any, bias_any, pad_any)
    return _CACHE[key]


def kernel(ori_x, x, x_mask,
           dw1, db1, pw1, pb1, dw2, db2, pw2, pb2,
           dw3, db3, pw3, pb3, dw4, db4, pw4, pb4,
           in_w, in_b, out_w, out_b, ffc_w, ffc_b, _results=None):
    ori_x = np.asarray(ori_x)
    x = np.asarray(x, dtype=np.float32)
    x_mask = np.asarray(x_mask)
    consts = build_host_consts(
        [np.asarray(d, np.float32) for d in (dw1, dw2, dw3, dw4)],
        [np.asarray(p, np.float32) for p in (pw1, pw2, pw3, pw4)],
        [np.asarray(d, np.float32) for d in (db1, db2, db3, db4)],
        [np.asarray(p, np.float32) for p in (pb1, pb2, pb3, pb4)],
        np.asarray(in_w, np.float32), np.asarray(in_b, np.float32),
        np.asarray(out_w, np.float32), np.asarray(out_b, np.float32),
        np.asarray(ffc_w, np.float32), np.asarray(ffc_b, np.float32))
    bias_any = any(np.any(np.asarray(v)) for v in
                   (db1, db2, db3, db4, pb1, pb2, pb3, pb4, in_b, out_b, ffc_b))
    mask_any = bool(np.asarray(x_mask).any())
    pad_any = bool((np.asarray(ori_x) == 0).any())
    nc = _get_program(consts, mask_any, bias_any, pad_any)

    xT = np.ascontiguousarray(x.transpose(0, 2, 1))       # [B, D, T]
    ori32 = ori_x.astype(np.int32)
    mask8 = x_mask.astype(np.uint8)
    in_maps = []
    for c in range(NC_):
        sl = slice(BS * c, BS * (c + 1))
        m = {"xT": xT[sl], "orix": ori32[sl], "xmask": mask8[sl]}
        m.update({k: v for k, v in consts.items() if not k.startswith("_")})
        in_maps.append(m)
    res = run_bass_kernel_spmd(nc, in_maps, list(range(NC_)))
    if _results is not None:
        _results.append(res)
    outT = np.concatenate([res.results[c]["out"] for c in range(NC_)], axis=0)
    return np.ascontiguousarray(outT.transpose(0, 2, 1)).astype(np.float32)


# revision 20
# speedup vs baseline: 1.0678x; 1.0534x over previous
"""Trainium2 Bass kernel for nn_Encoder_78649441124984.

Encoder: pos_emb + 4x(sepconv+res) + MHA(+res) + ffc(+res).
Sharding: data-parallel over batch, 8 cores x 4 batch elements, all
parameters replicated; no collectives.

On-device layout: activations kept transposed as [feature, time] tiles
([128, 512] SBUF tiles, feature on partitions).

Speed structure (v1):
 - sepconv (depthwise + pointwise) runs in fp8e4m3 with DoubleRow perf
   mode (2 contraction groups per pass at 0.5 cycles/row).  Weight
   quantization error is cancelled by a hi/lo split: W ~= fp8(W) +
   fp8(W - fp8(W)), two DoubleRow passes.  Depthwise pairs taps
   (s, s+4) as the two groups of one DoubleRow matmul over a
   zero-padded fp8 input tile.
 - everything else (qkv, scores, av, out/ffc proj) runs in
   float32r: at moving size >= 256 the PE runs f32r at bf16 speed,
   so this is free accuracy.  vaug/expt/anorm/owT run bf16 (their
   quantization is harmless); the residual stream stays f32r.
"""
import sys

sys.path.insert(0, "/opt/trn_rl_repo")

import numpy as np
import ml_dtypes

import concourse.bass as bass
import concourse.mybir as mybir
import concourse.tile as tile
from concourse import bacc
from concourse.ap import AP as APc
from concourse.bass_utils import run_bass_kernel_spmd

F32 = mybir.dt.float32
F32R = mybir.dt.float32r
BF16 = mybir.dt.bfloat16
FP8 = mybir.dt.float8e4
I32 = mybir.dt.int32
U8 = mybir.dt.uint8
AF = mybir.ActivationFunctionType
ALU = mybir.AluOpType
DR = mybir.MatmulPerfMode.DoubleRow
FP8NP = ml_dtypes.float8_e4m3
BF16NP = ml_dtypes.bfloat16

D = 500
H = 10
HD = 50
B, T = 32, 512
K = 7
NC_ = 8
BS = B // NC_          # batch shard per core
DP = 512               # padded feature dim
CT = 4                 # feature tiles (4 x 128 = 512 >= 500)
HP = 640               # padded head dim total (10 heads x 64 slots)
VW = 768               # v-proj rhs width (640 padded to 768 so the
                       # second psum piece has moving size 256)
XW = 520               # padded conv input tile width (4 + 512 + 4)


def _rows(ct):
    return min(128, D - 128 * ct)


def _head_col(h):
    return 128 * (h // 2) + 64 * (h % 2)


def _q8(a):
    return a.astype(FP8NP).astype(np.float32)


def build_host_consts(dw, pw, db, pb, in_w, in_b, out_w, out_b, ffc_w, ffc_b):
    """Pack all weights into device layouts. dw: [4][D,1,K], pw: [4][D,D]."""
    c = {}
    # ---------------- fp8 wall: depthwise diag pairs + pointwise ----------
    # depthwise: per (layer, block): 8 DoubleRow lhsT of [128, 2, 128]:
    # halves (hi, lo) x tap-pairs j=0..3 with taps (j-3, j+1); tap 4 = 0.
    # pointwise: per layer: halves (hi, lo) x ct-pairs c=0,1 of
    # [128, 2, 512]: group g holds pwT rows of ct=2c+g.
    w8_secs = []
    for l in range(4):
        dwf = dw[l][:, 0, :]                      # [D, K]
        dwp = np.zeros((DP, K + 2), np.float32)   # taps -3..3 plus zero tap 4
        dwp[:D, :K] = dwf
        hi = _q8(dwp)
        lo = dwp - hi                             # fp8 cast below
        diag = np.zeros((2, CT, 4, 128, 2, 128), np.float32)
        for half, w in ((0, hi), (1, lo)):
            for blk in range(CT):
                for j in range(4):
                    for g, tap in ((0, j), (1, j + 4)):
                        np.fill_diagonal(diag[half, blk, j, :, g, :],
                                         w[128 * blk:128 * blk + 128, tap])
        w8_secs.append((f"diag{l}", diag.reshape(2, CT, 4, 128, 256)
                        .transpose(3, 1, 0, 2, 4).reshape(128, -1)))
        # cols layout: blk-major, then half, then j, then [2x128]
        pwT = np.zeros((DP, DP), np.float32)
        pwT[:D, :D] = pw[l].T
        pwh = _q8(pwT)
        pwl = pwT - pwh
        pk = np.zeros((2, 2, 128, 2, DP), np.float32)
        for half, w in ((0, pwh), (1, pwl)):
            for cpair in range(2):
                for g in range(2):
                    ct = 2 * cpair + g
                    pk[half, cpair, :, g, :] = w[128 * ct:128 * ct + 128, :]
        # layout: half-major, then cpair, then [128, 2, 512] -> [128, 2048]
        w8_secs.append((f"pw{l}", pk.transpose(2, 0, 1, 3, 4).reshape(128, -1)))
    offs8 = {}
    w = 0
    parts = []
    for name, arr in w8_secs:
        offs8[name] = w
        w += arr.shape[1]
        parts.append(arr)
    c["wall8"] = np.concatenate(parts, 1).astype(FP8NP)
    c["_offs8"] = offs8

    # ---------------- f32r wall: qkv in-proj, v-proj, G ------------------
    scale = HD ** -0.5
    inwT = np.zeros((DP, 2 * HP), np.float32)
    inb_cols = np.zeros((128, 10), np.float32)
    for h in range(H):
        base = _head_col(h)
        qrows = slice(100 * (h // 2) + 50 * (h % 2),
                      100 * (h // 2) + 50 * (h % 2) + 50)
        inwT[:D, base:base + 50] = in_w[qrows, :].T * scale
        inb_cols[base % 128:base % 128 + 50, h // 2] = in_b[qrows] * scale
        krows = slice(500 + qrows.start, 500 + qrows.stop)
        inwT[:D, HP + base:HP + base + 50] = in_w[krows, :].T
        inb_cols[base % 128:base % 128 + 50, 5 + h // 2] = in_b[krows]
    wv = np.zeros((DP, VW), np.float32)
    crow = np.zeros((1, HP), np.float32)
    for h in range(H):
        base = _head_col(h)
        vrows = slice(1000 + 50 * h, 1000 + 50 * h + 50)
        wv[:D, base:base + 50] = in_w[vrows, :].T
        wv[D, base + 50] = 1.0    # ones column via the stream's 1.0 row
        crow[0, base:base + 50] = in_b[vrows]
        crow[0, base + 50] = 1.0
    G = np.zeros((5 * 128, H), np.float32)
    E = np.zeros((5 * H, 128), np.float32)
    for p in range(5):
        G[128 * p + 50, 2 * p] = 1.0
        G[128 * p + 114, 2 * p + 1] = 1.0
        E[H * p + 2 * p, 0:50] = 1.0
        E[H * p + 2 * p + 1, 64:114] = 1.0
    f32_secs = [("inwT", _rt(inwT)), ("wv", _rt(wv)),
                ("ffcT", _rt(np.pad(ffc_w.T, ((0, DP - D), (0, DP - D)))))]
    offsf = {}
    w = 0
    parts = []
    for name, arr in f32_secs:
        offsf[name] = w
        w += arr.shape[1]
        parts.append(arr)
    c["wallf"] = np.concatenate(parts, 1).astype(np.float32)
    c["_offsf"] = offsf

    # ---------------- bf16 wall: out-proj + ffc --------------------------
    owT = np.zeros((HP, DP), np.float32)
    for h in range(H):
        base = _head_col(h)
        owT[base:base + 50, :D] = out_w[:, 50 * h:50 * h + 50].T
    b16_secs = [("owT", _rt(owT)), ("G", _rt(G))]
    offsb = {}
    w = 0
    parts = []
    for name, arr in b16_secs:
        offsb[name] = w
        w += arr.shape[1]
        parts.append(arr)
    c["wallb"] = np.concatenate(parts, 1).astype(BF16NP)
    c["_offsb"] = offsb

    # ---------------- small f32 tensors ----------------------------------
    sm = np.concatenate(
        [inb_cols,
         np.pad(out_b, (0, DP - D)).reshape(CT, 128).T,
         np.pad(ffc_b, (0, DP - D)).reshape(CT, 128).T,
         np.concatenate([np.pad(db[l], (0, DP - D)).reshape(CT, 128).T
                         for l in range(4)], 1),
         np.concatenate([np.pad(pb[l], (0, DP - D)).reshape(CT, 128).T
                         for l in range(4)], 1)], 1).astype(np.float32)
    c["smallf"] = sm
    c["E_all"] = np.ascontiguousarray(
        np.concatenate([E[10 * p:10 * (p + 1), :] for p in range(5)], 1)
    ).astype(BF16NP)
    c["crow"] = crow.astype(np.float32)
    c["onesrow"] = np.ones((1, T), np.float32)
    half = D // 2
    inv = np.exp(np.arange(half, dtype=np.float64) * (-np.log(10000.0) / (half - 1)))
    pos = np.arange(1, T + 1, dtype=np.float64)
    ang = pos[None, :] * inv[:, None]
    peT = np.zeros((DP, T), np.float32)
    peT[:half, :] = np.sin(ang)
    peT[half:D, :] = np.cos(ang)
    c["peTp"] = _rt(peT).astype(BF16NP)
    return c


def _rt(a):
    """Repack row-tiled [n*128, C] -> [128, n*C] (tile ct at cols ct*C)."""
    n = a.shape[0] // 128
    return a.reshape(n, 128, a.shape[1]).transpose(1, 0, 2).reshape(128, -1)


def trace_program(consts, mask_any, bias_any, pad_any):
    """Build the SPMD Bass program (same for every core)."""
    nc = bacc.Bacc("TRN2", target_bir_lowering=False, debug=False,
                   num_devices=NC_)

    import os
    dbg = os.environ.get("BASSDBG") == "1"
    xT_d = nc.dram_tensor("xT", [BS, D, T], F32, kind="ExternalInput")
    orix_d = nc.dram_tensor("orix", [BS, T], I32, kind="ExternalInput")
    xmask_d = nc.dram_tensor("xmask", [BS, T], U8, kind="ExternalInput")
    out_d = nc.dram_tensor("out", [BS, D, T], F32, kind="ExternalOutput")
    dbg_d = None
    if dbg:
        dbg_d = {}
        for n in ("pos", "l1", "l4", "qk0", "qk5", "x2"):
            dbg_d[n] = nc.dram_tensor(f"dbg_{n}", [512, T], F32R, kind="ExternalOutput")
        for n in ("vaug", "abuf", "anorm", "mbc"):
            dbg_d[n] = nc.dram_tensor(f"dbg_{n}", [512, T], BF16, kind="ExternalOutput")

    wd = {"_offs8": consts["_offs8"], "_offsf": consts["_offsf"],
          "_offsb": consts["_offsb"]}
    dts = {"wall8": FP8, "wallf": F32R, "wallb": BF16, "smallf": F32,
           "E_all": BF16, "crow": F32, "peTp": BF16, "onesrow": F32R}
    for name, arr in consts.items():
        if name.startswith("_"):
            continue
        wd[name] = nc.dram_tensor(name, list(arr.shape), dts[name],
                                  kind="ExternalInput")

    with tile.TileContext(nc, num_cores=NC_) as tc:
        wd["_dbg"] = dbg_d
        _trace_body(nc, tc, wd, xT_d, orix_d, xmask_d, out_d,
                    mask_any, bias_any, pad_any)
    nc.finalize()
    return nc


def _trace_body(nc, tc, wd, xT_d, orix_d, xmask_d, out_d,
                mask_any, bias_any, pad_any):
    from contextlib import ExitStack
    ctx = ExitStack()
    with ctx:
        wpool = ctx.enter_context(tc.tile_pool(name="w", bufs=1))
        offs8 = wd["_offs8"]
        offsf = wd["_offsf"]
        offsb = wd["_offsb"]
        W8 = wd["wall8"].shape[1]
        WF = wd["wallf"].shape[1]
        WB = wd["wallb"].shape[1]
        wall8 = wpool.tile([128, W8], FP8, tag="wall8", name="wall8")
        wallf = wpool.tile([128, WF], F32R, tag="wallf", name="wallf")
        wallb = wpool.tile([128, WB], BF16, tag="wallb", name="wallb")
        # urgent small constants on the ACT ring
        peTp = wpool.tile([128, CT * T], BF16, tag="peTp", name="peTp")
        nc.scalar.dma_start(peTp[:], wd["peTp"][:])
        smallf = wpool.tile([128, 50], F32, tag="smallf", name="smallf")
        nc.scalar.dma_start(smallf[:], wd["smallf"][:])
        E_t = wpool.tile([H, 5 * 128], BF16, tag="E_t", name="E_t")
        nc.scalar.dma_start(E_t[:], wd["E_all"][:])
        crow_t = None
        if bias_any:
            crow_t = wpool.tile([1, HP], F32, tag="crow", name="crow")
            nc.scalar.dma_start(crow_t[:], wd["crow"][:])
        # walls on the SP ring, section-by-section in first-use order
        def sec_dmas(tile_t, dram, offd, order, width):
            sw = {}
            for s in offd:
                nxt = [offd[t] for t in offd if offd[t] > offd[s]]
                sw[s] = (min(nxt) if nxt else width) - offd[s]
            for s in order:
                nc.sync.dma_start(tile_t[:, offd[s]:offd[s] + sw[s]],
                                  dram[:, offd[s]:offd[s] + sw[s]])
        o8 = []
        for l in range(4):
            o8 += [f"diag{l}", f"pw{l}"]
        sec_dmas(wall8, wd["wall8"], offs8, o8, W8)
        sec_dmas(wallf, wd["wallf"], offsf, ["inwT", "wv", "ffcT"], WF)
        sec_dmas(wallb, wd["wallb"], offsb, ["owT", "G"], WB)
        C_t = None
        if bias_any:
            C_t = wpool.tile([128, HP], F32, tag="C", name="C")
            nc.gpsimd.partition_broadcast(C_t[:], crow_t[:])

        # weight-slice helpers -------------------------------------------
        def dw_lhsT(l, blk, half, j):
            off = offs8[f"diag{l}"] + blk * 2048 + half * 1024 + j * 256
            base = wall8[:, off:off + 256]
            return APc(base.tensor, base.offset,
                       [list(base.ap[0]), [128, 2], [1, 128]])

        def pw_lhsT(l, half, cpair, ot):
            off = offs8[f"pw{l}"] + half * 2048 + cpair * 1024 + 128 * ot
            base = wall8[:, off:off + 1]
            return APc(base.tensor, base.offset,
                       [list(base.ap[0]), [512, 2], [1, 128]])

        inwT = [wallf[:, offsf["inwT"] + 2 * HP * ct:
                       offsf["inwT"] + 2 * HP * (ct + 1)] for ct in range(CT)]
        wv = [wallf[:, offsf["wv"] + VW * ct:offsf["wv"] + VW * (ct + 1)]
              for ct in range(CT)]
        G = [wallb[:, offsb["G"] + H * p:offsb["G"] + H * (p + 1)]
             for p in range(5)]
        owT = [wallb[:, offsb["owT"] + DP * p:offsb["owT"] + DP * (p + 1)]
               for p in range(5)]
        ffcT = [wallf[:, offsf["ffcT"] + DP * ct:offsf["ffcT"] + DP * (ct + 1)]
                for ct in range(CT)]
        E = [E_t[:, 128 * p:128 * (p + 1)] for p in range(5)]
        peT = [peTp[:, T * ct:T * (ct + 1)] for ct in range(CT)]
        inb_cols = smallf[:, 0:10]
        outb_col = smallf[:, 10:14]
        ffcb_col = smallf[:, 14:18]
        db_cols = [smallf[:, 18 + CT * l:18 + CT * (l + 1)] for l in range(4)]
        pb_cols = [smallf[:, 34 + CT * l:34 + CT * (l + 1)] for l in range(4)]

        # ---- per-batch-element pools ----
        xpool = ctx.enter_context(tc.tile_pool(name="x", bufs=3))
        f8pool = ctx.enter_context(tc.tile_pool(name="f8", bufs=2))
        mpool = ctx.enter_context(tc.tile_pool(name="m", bufs=2))
        qkpool = ctx.enter_context(tc.tile_pool(name="qk", bufs=1))
        epool = ctx.enter_context(tc.tile_pool(name="e", bufs=2))
        apool = ctx.enter_context(tc.tile_pool(name="a", bufs=2))
        opool = ctx.enter_context(tc.tile_pool(name="o", bufs=2))
        pp = ctx.enter_context(tc.tile_pool(name="pp", bufs=6, space="PSUM"))
        pa = ctx.enter_context(tc.tile_pool(name="pa", bufs=1, space="PSUM"))

        gens = [
            _trace_batch(nc, tc, b, wd, xT_d, orix_d, xmask_d, out_d,
                         dw_lhsT, pw_lhsT, inwT, wv, owT, ffcT, peT, G, E, C_t,
                         inb_cols, outb_col, ffcb_col, db_cols, pb_cols,
                         xpool, f8pool, mpool, qkpool, epool, apool, opool,
                         pp, pa, mask_any, bias_any, pad_any)
            for b in range(BS)
        ]
        done = [False] * BS
        last = ["f"] * BS

        def step(i):
            try:
                last[i] = next(gens[i])
            except StopIteration:
                done[i] = True

        import os
        if os.environ.get("NOILV") == "1":
            for g in gens:
                for _ in g:
                    pass
        else:
            while not done[0] and last[0] == "f":
                step(0)
            for b in range(BS):
                nxt = b + 1 if b + 1 < BS else None
                while not done[b]:
                    step(b)
                    if nxt is not None and not done[nxt] and last[nxt] == "f":
                        step(nxt)


def _dw_rhs(xf8, j):
    """Depthwise moving AP: tap pair (j-3, j+1) as two gap-4 groups over a
    [128, 520] zero-padded fp8 tile (data at cols 4..516)."""
    base = xf8[:, 0:512]
    return APc(base.tensor, base.offset + 1 + j,
               [list(base.ap[0]), [4, 2], [1, 512]])


def _pw_rhs(dwout8, cpair):
    """Pointwise moving AP: blocks (2c, 2c+1) of the [128, 2048] fp8 dwout
    tile as the two groups."""
    base = dwout8[:, 0:512]
    return APc(base.tensor, base.offset + 1024 * cpair,
               [list(base.ap[0]), [512, 2], [1, 512]])


def _dump(nc, wd, b, name, tiles, rows=128):
    dbg = wd.get("_dbg")
    import os
    if dbg is None or b != int(os.environ.get("BASSDBG_B", "0")) or name not in dbg:
        return
    for i, t in enumerate(tiles):
        nc.sync.dma_start(wd["_dbg"][name][128 * i:128 * i + rows, :],
                          t[0:rows, :] if rows < 128 else t[:])


def _trace_batch(nc, tc, b, wd, xT_d, orix_d, xmask_d, out_d,
                 dw_lhsT, pw_lhsT, inwT, wv, owT, ffcT, peT, G, E, C_t,
                 inb_cols, outb_col, ffcb_col, db_cols, pb_cols,
                 xpool, f8pool, mpool, qkpool, epool, apool, opool,
                 pp, pa, mask_any, bias_any, pad_any):
    # ---------------- pos_emb + input load ----------------
    if pad_any:
        mrow = mpool.tile([1, T], I32, tag="mrow_i", name="mrow_i")
        nc.scalar.dma_start(mrow[:], orix_d[b:b + 1, :])
        mrow_f = mpool.tile([1, T], F32, tag="mrow_f", name="mrow_f")
        nc.vector.tensor_copy(mrow_f[:], mrow[:])
        nc.vector.tensor_scalar_min(mrow_f[:], mrow_f[:], 1.0)
        dbgd = wd.get("_dbg")
        import os as _os
        if dbgd is not None and b == int(_os.environ.get("BASSDBG_B", "0")):
            nc.sync.dma_start(dbgd["mbc"][1:2, :], mrow_f[:])
        m_bc = mpool.tile([128, T], F32, tag="m_bc", name="m_bc", bufs=1)
        nc.gpsimd.partition_broadcast(m_bc[:], mrow_f[:])
        _dump(nc, wd, b, "mbc", [m_bc])
    xin = [xpool.tile([128, T], F32, tag=f"xin{ct}", name=f"xin{ct}", bufs=1)
           for ct in range(CT)]
    for ct in range(CT):
        r = _rows(ct)
        if r < 128:
            nc.gpsimd.memset(xin[ct][96:128, :], 0.0)
        nc.scalar.dma_start(xin[ct][0:r, :], xT_d[b, 128 * ct:128 * ct + r, :])
    xcur = [xpool.tile([128, T], F32R, tag=f"x{ct}", name=f"x{ct}") for ct in range(CT)]
    if pad_any:
        for ct in range(CT):
            pem = mpool.tile([128, T], F32, tag="pem", name="pem", bufs=1)
            nc.vector.tensor_tensor(pem[:], peT[ct][:], m_bc[:], op=ALU.mult)
            nc.vector.tensor_tensor(xcur[ct][:], xin[ct][:], pem[:], op=ALU.add)
    else:
        for ct in range(CT):
            nc.vector.tensor_tensor(xcur[ct][:], xin[ct][:], peT[ct][:],
                                    op=ALU.add)
    nc.scalar.dma_start(xcur[3][116:117, :], wd["onesrow"][:])
    _dump(nc, wd, b, "pos", xcur)

    yield "f"
    # ---------------- 4x sepconv + residual ----------------
    for l in range(4):
        # fp8 conv input tiles, zero-padded borders (cols 0:4 and 516:520)
        xf8 = []
        for ct in range(CT):
            t = f8pool.tile([128, XW], FP8, tag=f"xf8_{ct}", name=f"xf8_{ct}")
            bord = APc(t[:, 0:1].tensor, t[:, 0:1].offset,
                       [list(t[:, 0:1].ap[0]), [516, 2], [1, 4]])
            nc.gpsimd.memset(bord, 0.0)
            nc.gpsimd.tensor_copy(t[:, 4:516], xcur[ct][:])
            xf8.append(t)
        dwout8 = f8pool.tile([128, 2048], FP8, tag="dwout8", name="dwout8")
        for blk in range(CT):
            pdw = pp.tile([128, T], F32, tag="ps", name="ps")
            first = True
            for half in range(2):
                for j in range(4):
                    nc.tensor.matmul(pdw[:], dw_lhsT(l, blk, half, j),
                                     _dw_rhs(xf8[blk], j),
                                     start=first, stop=(half == 1 and j == 3),
                                     perf_mode=DR, skip_group_check=True)
                    first = False
            if bias_any:
                nc.scalar.activation(dwout8[:, 512 * blk:512 * (blk + 1)],
                                     pdw[:], AF.Identity,
                                     bias=db_cols[l][:, blk:blk + 1])
            else:
                nc.scalar.activation(dwout8[:, 512 * blk:512 * (blk + 1)],
                                     pdw[:], AF.Identity)
            if blk == 1:
                yield "f"
        yield "f"
        xnext = [xpool.tile([128, T], F32R, tag=f"x{ot}", name=f"x{ot}") for ot in range(CT)]
        for ot in range(CT):
            ppw = pp.tile([128, T], F32, tag="ps", name="ps")
            first = True
            for half in range(2):
                for cpair in range(2):
                    nc.tensor.matmul(ppw[:], pw_lhsT(l, half, cpair, ot),
                                     _pw_rhs(dwout8, cpair),
                                     start=first,
                                     stop=(half == 1 and cpair == 1),
                                     perf_mode=DR, skip_group_check=True)
                    first = False
            if bias_any:
                nc.vector.scalar_tensor_tensor(xnext[ot][:], ppw[:],
                                               pb_cols[l][:, ot:ot + 1],
                                               xcur[ot][:],
                                               op0=ALU.add, op1=ALU.add)
            else:
                nc.vector.tensor_tensor(xnext[ot][:], ppw[:], xcur[ot][:],
                                        op=ALU.add)
        xcur = xnext
        if l == 0:
            _dump(nc, wd, b, "l1", xcur)
        if l == 3:
            _dump(nc, wd, b, "l4", xcur)
        yield "f"

    # ---------------- attention ----------------
    # q (p=0..4) and k (p=5..9) pair tiles, f32r
    qk = []
    for p in range(10):
        pq = pp.tile([128, T], F32, tag="ps", name="ps")
        for ct in range(CT):
            nc.tensor.matmul(pq[:], inwT[ct][:, 128 * p:128 * (p + 1)],
                             xcur[ct][:], start=(ct == 0), stop=(ct == CT - 1))
        qt = qkpool.tile([128, T], F32R, tag=f"qk{p}", name=f"qk{p}")
        if bias_any:
            nc.scalar.activation(qt[:], pq[:], AF.Identity,
                                 bias=inb_cols[:, p:p + 1])
        else:
            nc.scalar.activation(qt[:], pq[:], AF.Identity)
        qk.append(qt)
        if p == 0:
            _dump(nc, wd, b, "qk0", [qt])
        if p == 5:
            _dump(nc, wd, b, "qk5", [qt])
        if p % 3 == 2:
            yield "b"
    # v^T (+ ones column): per kt: [128, 512] + [128, 256] psum pieces
    vaug = []
    for kt in range(CT):
        pv0 = pp.tile([128, T], F32, tag="ps", name="ps")
        pv1 = pp.tile([128, VW - T], F32, tag="ps", name="ps")
        for ct in range(CT):
            nc.tensor.matmul(pv0[:], xcur[ct][:, 128 * kt:128 * (kt + 1)],
                             wv[ct][:, 0:512], start=(ct == 0), stop=(ct == CT - 1))
            nc.tensor.matmul(pv1[:], xcur[ct][:, 128 * kt:128 * (kt + 1)],
                             wv[ct][:, 512:VW], start=(ct == 0), stop=(ct == CT - 1))
        vt = qkpool.tile([128, HP], BF16, tag=f"vaug{kt}", name=f"vaug{kt}", bufs=2)
        if bias_any:
            nc.vector.tensor_tensor(vt[:, 0:512], pv0[:], C_t[:, 0:512], op=ALU.add)
            nc.vector.tensor_tensor(vt[:, 512:HP], pv1[:, 0:128], C_t[:, 512:HP],
                                    op=ALU.add)
        else:
            nc.scalar.activation(vt[:, 0:512], pv0[:], AF.Identity)
            nc.scalar.activation(vt[:, 512:HP], pv1[:, 0:128], AF.Identity)
        vaug.append(vt)
        if kt == 0:
            _dump(nc, wd, b, "vaug", [vt[:, 0:512]])
        if kt % 2 == 1:
            yield "b"
    keep = None
    if mask_any:
        keep = []
        for kt in range(CT):
            kc_u8 = mpool.tile([128, 1], U8, tag=f"kc8_{kt}", name=f"kc8_{kt}")
            nc.sync.dma_start(
                kc_u8[:],
                xmask_d[b, 128 * kt:128 * (kt + 1)].rearrange(
                    "(t one) -> t one", one=1))
            kc = mpool.tile([128, 1], F32, tag=f"kc{kt}", name=f"kc{kt}")
            nc.vector.tensor_copy(kc[:], kc_u8[:])
            nc.vector.tensor_scalar(kc[:], kc[:], -1.0, 1.0,
                                    op0=ALU.mult, op1=ALU.add)
            keep.append(kc)

    abuf = []
    for p in range(5):
        pat = pa.tile([128, T], F32, tag="pat", name="pat", bufs=2)
        for h in (2 * p, 2 * p + 1):
            s = 64 * (h % 2)
            expt = []
            for m in range(CT):
                ps_ = pp.tile([128, T], F32, tag="ps", name="ps")
                nc.tensor.matmul(ps_[:], qk[5 + p][s:s + 64, 128 * m:128 * (m + 1)],
                                 qk[p][s:s + 64, :], start=True, stop=True)
                et = epool.tile([128, T], BF16, tag="exp", name="exp", bufs=6)
                nc.scalar.activation(et[:], ps_[:], AF.Exp)
                if keep is not None:
                    nc.vector.tensor_scalar_mul(et[:], et[:], keep[m][:])
                expt.append(et)
            yield "b"
            for m in range(CT):
                nc.tensor.matmul(pat[s:s + 64, :],
                                 vaug[m][:, 128 * p + s:128 * p + s + 64],
                                 expt[m][:], start=(m == 0), stop=(m == CT - 1))
        ab = apool.tile([128, T], BF16, tag=f"abuf{p}", name=f"abuf{p}", bufs=1)
        nc.vector.tensor_copy(ab[:], pat[:])
        abuf.append(ab)
        if p == 0:
            _dump(nc, wd, b, "abuf", [ab])
        yield "b"
    pr = pp.tile([H, T], F32, tag="ps", name="ps")
    for p in range(5):
        nc.tensor.matmul(pr[:], G[p][:], abuf[p][:],
                         start=(p == 0), stop=(p == 4))
    rrec = apool.tile([H, T], BF16, tag="rrec", name="rrec", bufs=1)
    with nc.allow_low_precision(reason="softmax recip; normalized weights"):
        nc.vector.reciprocal(rrec[:], pr[:])
    yield "b"
    anorm = []
    for p in range(5):
        pbc = pp.tile([128, T], F32, tag="ps", name="ps")
        nc.tensor.matmul(pbc[:], E[p][:], rrec[:], start=True, stop=True)
        an = apool.tile([128, T], BF16, tag=f"anorm{p}", name=f"anorm{p}", bufs=1)
        nc.vector.tensor_tensor(an[:], abuf[p][:], pbc[:], op=ALU.mult)
        anorm.append(an)
    _dump(nc, wd, b, "anorm", [anorm[0]])
    # out-proj + residual
    x2 = [xpool.tile([128, T], F32R, tag=f"x{ot}", name=f"x{ot}") for ot in range(CT)]
    for ot in range(CT):
        po = pp.tile([128, T], F32, tag="ps", name="ps")
        for p in range(5):
            nc.tensor.matmul(po[:], owT[p][:, 128 * ot:128 * (ot + 1)],
                             anorm[p][:], start=(p == 0), stop=(p == 4))
        if bias_any:
            nc.vector.scalar_tensor_tensor(x2[ot][:], po[:],
                                           outb_col[:, ot:ot + 1], xcur[ot][:],
                                           op0=ALU.add, op1=ALU.add)
        else:
            nc.vector.tensor_tensor(x2[ot][:], po[:], xcur[ot][:], op=ALU.add)
    _dump(nc, wd, b, "x2", x2)
    yield "b"

    # ---------------- ffc + residual + store ----------------
    for ot in range(CT):
        pf = pp.tile([128, T], F32, tag="ps", name="ps")
        for ct in range(CT):
            nc.tensor.matmul(pf[:], ffcT[ct][:, 128 * ot:128 * (ot + 1)],
                             x2[ct][:], start=(ct == 0), stop=(ct == CT - 1))
        ott = opool.tile([128, T], F32, tag=f"out{ot}", name=f"out{ot}", bufs=1)
        if bias_any:
            nc.vector.scalar_tensor_tensor(ott[:], pf[:],
                                           ffcb_col[:, ot:ot + 1], x2[ot][:],
                                           op0=ALU.add, op1=ALU.add)
        else:
            nc.vector.tensor_tensor(ott[:], pf[:], x2[ot][:], op=ALU.add)
        r = _rows(ot)
        nc.sync.dma_start(out_d[b, 128 * ot:128 * ot + r, :], ott[0:r, :])


_CACHE = {}


def _get_program(consts, mask_any, bias_any, pad_any):
    key = (mask_any, bias_any, pad_any)
    if key not in _CACHE:
        _CACHE[key] = trace_program(consts, mask_any, bias_any, pad_any)
    return _CACHE[key]


def kernel(ori_x, x, x_mask,
           dw1, db1, pw1, pb1, dw2, db2, pw2, pb2,
           dw3, db3, pw3, pb3, dw4, db4, pw4, pb4,
           in_w, in_b, out_w, out_b, ffc_w, ffc_b, _results=None):
    ori_x = np.asarray(ori_x)
    x = np.asarray(x, dtype=np.float32)
    x_mask = np.asarray(x_mask)
    consts = build_host_consts(
        [np.asarray(d, np.float32) for d in (dw1, dw2, dw3, dw4)],
        [np.asarray(p, np.float32) for p in (pw1, pw2, pw3, pw4)],
        [np.asarray(d, np.float32) for d in (db1, db2, db3, db4)],
        [np.asarray(p, np.float32) for p in (pb1, pb2, pb3, pb4)],
        np.asarray(in_w, np.float32), np.asarray(in_b, np.float32),
        np.asarray(out_w, np.float32), np.asarray(out_b, np.float32),
        np.asarray(ffc_w, np.float32), np.asarray(ffc_b, np.float32))
    bias_any = any(np.any(np.asarray(v)) for v in
                   (db1, db2, db3, db4, pb1, pb2, pb3, pb4, in_b, out_b, ffc_b))
    mask_any = bool(np.asarray(x_mask).any())
    pad_any = bool((np.asarray(ori_x) == 0).any())
    nc = _get_program(consts, mask_any, bias_any, pad_any)

    xT = np.ascontiguousarray(x.transpose(0, 2, 1))       # [B, D, T]
    ori32 = ori_x.astype(np.int32)
    mask8 = x_mask.astype(np.uint8)
    in_maps = []
    for c in range(NC_):
        sl = slice(BS * c, BS * (c + 1))
        m = {"xT": xT[sl], "orix": ori32[sl], "xmask": mask8[sl]}
        m.update({k: v for k, v in consts.items() if not k.startswith("_")})
        in_maps.append(m)
    res = run_bass_kernel_spmd(nc, in_maps, list(range(NC_)))
    if _results is not None:
        _results.append(res)
    outT = np.concatenate([res.results[c]["out"] for c in range(NC_)], axis=0)
    return np.ascontiguousarray(outT.transpose(0, 2, 1)).astype(np.float32)
